# revision 2
# baseline (speedup 1.0000x reference)
"""Trainium2 Bass kernel for nn_AutoSparseLinear: out = sparse @ weight + b.

Shapes (hardcoded): sparse [4096, 4096] f32, weight [4096, 4096] f32,
b [4096] f32 -> out [4096, 4096] f32.

Strategy: 2D shard across 8 cores as 4 batch-shards x 2 column-shards.
Core c = 4*cs + br computes out[br*1024:(br+1)*1024, cs*2048:(cs+1)*2048].
This cuts per-core HBM traffic to x 8.4 MiB (resident) + W 16.8 MiB
(streamed once) + out 4.2 MiB, vs ~41.5 MiB for pure batch sharding -
on this part DMA time adds to PE time, so fewer streamed bytes wins.

Per core: out_shard^T = Wshard^T @ xshard^T on the PE with W tiles
stationary (streamed from HBM exactly once) and x^T SBUF-resident as the
moving operand. Operands are cast to fp16 on the host: fp16 runs the PE
at 1 cycle/row and halves DMA traffic vs fp32; fp8 cannot pass the 2e-2
error gate (e4m3 x-side quantization alone is 2.2e-2) so 16-bit is the
compute floor. PSUM accumulation stays fp32; 16 n-tiles x 2 m-halves x
32 k-tiles = 1024 matmuls of 512 moving columns = ~218 us PE floor.
Bias is added during PSUM->SBUF eviction on the vector engine ([128,1]
per-partition scalar broadcast); the eviction writes bf16, halving the
output-store traffic, total absmax rel err ~2.3e-3 (budget 2e-2).

Ramp tuning: x is loaded in 8 chunks (1 MiB) issued before bias, and the
first two W tiles are split into quarter-DMAs so the first matmul starts
after ~0.5 MiB of DMA instead of ~2 MiB. W streams on the scalar (ACT)
HWDGE ring; x, bias and output stores use the sync (SP) ring so the
first W tile never queues behind the x load.

Host side only reshapes/transposes/casts for layout and reassembles the
output block grid.
"""

import numpy as np

import concourse.bass as bass
import concourse.mybir as mybir
import concourse.tile as tile
from concourse import bacc
from concourse.bass_utils import run_bass_kernel_spmd

P = 128
B = 4096
NCORES = 8
BR = 4            # batch shards
CS = 2            # column shards
MB = B // BR      # 1024 batch rows per core
MH = 512          # moving columns per matmul
NMH = MB // MH    # 2 m-halves
K = 4096
N = 4096
NC_ = N // CS     # 2048 out features per core
KT = K // P       # 32
NT = NC_ // P     # 16 n-tiles per core
XCH = 8           # x chunks (by k-range)
KPC = KT // XCH   # 4 k-tiles per x chunk
WSPLIT = 4
NSPLIT = 2

MM_DT = mybir.dt.float16
NP_DT = np.float16
OUT_DT = mybir.dt.bfloat16

_CACHE = {}


def build_nc(repeat=1):
    nc = bacc.Bacc("TRN2", target_bir_lowering=False, debug=False)

    # xT[p, kt*MB + m] = x_shard[m, kt*P + p]
    xT = nc.dram_tensor("xT", [P, KT * MB], MM_DT, kind="ExternalInput").ap()
    # w[nt, p, kt*P + j] = Wshard[kt*P + p, nt*P + j]
    w = nc.dram_tensor("w", [NT, P, KT * P], MM_DT, kind="ExternalInput").ap()
    # bias[p, nt] = b_shard[nt*P + p]
    bias = nc.dram_tensor("bias", [P, NT], mybir.dt.float32,
                          kind="ExternalInput").ap()
    # outT[nt, p, m] = out_shard[m, nt*P + p]
    outT = nc.dram_tensor("outT", [NT, P, MB], OUT_DT,
                          kind="ExternalOutput").ap()

    with tile.TileContext(nc) as tc:
        with (
            tc.tile_pool(name="xpool", bufs=1) as xpool,
            tc.tile_pool(name="wpool", bufs=6) as wpool,
            tc.tile_pool(name="opool", bufs=4) as opool,
            tc.tile_pool(name="bpool", bufs=1) as bpool,
            tc.tile_pool(name="pspool", bufs=4, space="PSUM") as pspool,
        ):
            xch = []
            for c in range(XCH):
                xc = xpool.tile([P, KPC * MB], MM_DT, name=f"xc{c}",
                                tag=f"xc{c}")
                nc.sync.dma_start(xc[:],
                                  xT[:, c * KPC * MB:(c + 1) * KPC * MB])
                xch.append(xc)

            bt = bpool.tile([P, NT], mybir.dt.float32)
            nc.sync.dma_start(bt[:], bias[:])

            def xslice(kt, mh):
                c, j = divmod(kt, KPC)
                return xch[c][:, (j * NMH + mh) * MH:(j * NMH + mh + 1) * MH]

            for r in range(repeat):
                for nt in range(NT):
                    wt = wpool.tile([P, KT * P], MM_DT, name=f"wt{r}_{nt}",
                                    tag="wt")
                    if r == 0 and nt < NSPLIT:
                        kq = KT // WSPLIT
                        for q in range(WSPLIT):
                            nc.scalar.dma_start(
                                wt[:, q * kq * P:(q + 1) * kq * P],
                                w[nt][:, q * kq * P:(q + 1) * kq * P])
                    else:
                        nc.scalar.dma_start(wt[:], w[nt])
                    for mh in range(NMH):
                        ps = pspool.tile([P, MH], mybir.dt.float32,
                                         name=f"ps{r}_{nt}_{mh}", tag="ps")
                        for kt in range(KT):
                            nc.tensor.matmul(
                                ps[:],
                                wt[:, kt * P:(kt + 1) * P],
                                xslice(kt, mh),
                                start=(kt == 0),
                                stop=(kt == KT - 1),
                            )
                        ot = opool.tile([P, MH], OUT_DT,
                                        name=f"ot{r}_{nt}_{mh}", tag="ot")
                        nc.vector.tensor_scalar_add(ot[:], ps[:],
                                                    bt[:, nt:nt + 1])
                        nc.sync.dma_start(outT[nt][:, mh * MH:(mh + 1) * MH],
                                          ot[:])

    nc.compile()
    return nc


def get_nc():
    if "nc" not in _CACHE:
        _CACHE["nc"] = build_nc()
    return _CACHE["nc"]


def shard_inputs(sparse, weight, b):
    sparse = np.asarray(sparse)
    weight = np.asarray(weight).astype(NP_DT)
    b = np.ascontiguousarray(np.asarray(b), dtype=np.float32)

    wshards = []
    bshards = []
    for cs in range(CS):
        ws = weight[:, cs * NC_:(cs + 1) * NC_]
        wb = np.ascontiguousarray(
            ws.reshape(KT, P, NT, P).transpose(2, 1, 0, 3)
            .reshape(NT, P, KT * P)
        )
        wshards.append(wb)
        bs = b[cs * NC_:(cs + 1) * NC_]
        bshards.append(np.ascontiguousarray(bs.reshape(NT, P).T))

    in_maps = []
    for c in range(NCORES):
        cs, br = divmod(c, BR)
        xs = sparse[br * MB:(br + 1) * MB, :].astype(NP_DT)
        # xT[p, (kt*NMH + mh)*MH + m] = xs[mh*MH + m, kt*P + p]
        xb = np.ascontiguousarray(
            xs.reshape(NMH, MH, KT, P).transpose(3, 2, 0, 1)
            .reshape(P, KT * MB)
        )
        in_maps.append({"xT": xb, "w": wshards[cs], "bias": bshards[cs]})
    return in_maps


def unshard_output(results):
    out = np.empty((B, N), dtype=np.float32)
    for c in range(NCORES):
        cs, br = divmod(c, BR)
        oT = results[c]["outT"].astype(np.float32)  # [NT, P, MB]
        out[br * MB:(br + 1) * MB, cs * NC_:(cs + 1) * NC_] = \
            oT.reshape(NC_, MB).T
    return np.ascontiguousarray(out)


def kernel(sparse, weight, b, **run_kwargs):
    nc = get_nc()
    in_maps = shard_inputs(sparse, weight, b)
    res = run_bass_kernel_spmd(nc, in_maps, core_ids=list(range(NCORES)),
                               **run_kwargs)
    out = unshard_output(res.results)
    if run_kwargs:
        _CACHE["last_result"] = res
    return out
